# revision 39
# baseline (speedup 1.0000x reference)
"""DeChunk layer (ragged sequence scan + segment broadcast) on 8 trn2 NeuronCores.

Math (see reference): per batch b
  - boundary tokens (stable-sorted to front) give p_m for m < K
  - dt = bf16(log(1/(1-p))), x = bf16(hidden/dt)
  - linear recurrence h_m = exp(-dt_m) h_{m-1} + dt_m p_m x_m over the M sorted slots
  - out[l] = f32(bf16(h_{c(l)})), c(l) = cumsum(boundary_mask)-1  (segment broadcast)

Only scan positions m < K are ever read by the output, and the recurrence is
causal, so the tail is skipped.  The scan is reformulated as chunked matmuls
(Mamba-style): within a 127-token chunk with inclusive prefix sums S_m,
  h_m = exp(-S_m) h_in + sum_{j<=m} p_j exp(S_j - S_m) hidden_j
(the reference's w_j = dt_j p_j weight divided by its 1/dt_j x-scaling), which
is one [128,128]^T @ [128,512] matmul per chunk.  Contraction row 0 carries
h_in; lhsT column 0 duplicates the last token's column so the chunk-final
state lands on PSUM partition 0, partition-aligned with the next chunk's h_in
slot (engine ops cannot cross partitions and must start at partition 0).

Sharding: core k handles batch k//2, d_model half k%2 (512 columns).  Index
work (sort/cumsum/prefix sums/triangular exp matrices; all tiny) is done on
host; the device does all heavy data movement and math: casting DMA loads
(f32->bf16), 33 chunk matmuls + carried-state chain, bf16 h bounce to DRAM,
indirect-DMA row gather (the segment broadcast), and a widening casting DMA
(bf16->f32) to the output.

Written in raw Bass (explicit semaphores, no TileContext): this container's
walrus codegen supports at most one attached sync-wait per compute
instruction, which rules out Tile's scheduler; raw Bass emits waits as
standalone instructions.
"""

import numpy as np
import ml_dtypes

import concourse.bass as bass
import concourse.bacc as bacc
import concourse.mybir as mybir
from concourse.bass_utils import run_bass_kernel_spmd

EPS = 1e-4
B, L, M, D = 4, 8192, 4096, 1024
NCORES = 8
D2 = D // 2            # columns per core
P = 128
C = 127                # tokens per scan chunk (slot 0 = carried state)
NCH = (M + C - 1) // C # 33
MP = NCH * C           # 4191
XG = 4                 # scan chunks per load/store group
NG = (NCH + XG - 1) // XG  # 9
NQ = 8                 # output expansion calls
RG = L // NQ           # 1024 output rows per call
G = RG // P            # 8 rows per partition per call
NPS = 4                # PSUM buffers

bf16 = ml_dtypes.bfloat16
f32 = mybir.dt.float32
bf16_dt = mybir.dt.bfloat16
i32 = mybir.dt.int32
i16 = mybir.dt.int16


def _ng(g: int) -> int:
    return min(XG, NCH - g * XG)


def _n_hw(q: int) -> int:
    """h-write groups required before output-gather q (c(l) <= l bound)."""
    bound = min((q + 1) * RG, MP)
    return -(-bound // (XG * C))


def _build_program() -> bass.Bass:
    # Bacc (not plain Bass): its compile() pipeline lowers the gpsimd
    # library-reload pseudo-instruction and custom-ISA ops for walrus
    nc = bacc.Bacc("TRN2")
    x_in = nc.declare_dram_parameter("x", [NCH, C, D2], f32, isOutput=False)
    lt_in = nc.declare_dram_parameter("lt", [NCH, P, P], bf16_dt, isOutput=False)
    # int16 gather indices, dma_gather layout: position i of call q lives at
    # [16k + i%16, q*(RG//16) + i//16] for every Q7 core k
    pb_in = nc.declare_dram_parameter("pb", [P, NQ * (RG // 16)], i16, isOutput=False)
    out = nc.declare_dram_parameter("out", [L, D2], f32, isOutput=True)
    h_dram = nc.dram_tensor("h_dram", [MP, D2], bf16_dt)

    from contextlib import ExitStack

    with ExitStack() as es:
        rhs = [
            es.enter_context(nc.sbuf_tensor(f"rhs{i}", [P, XG * D2], bf16_dt))
            for i in range(2)
        ]
        lt = [
            es.enter_context(nc.sbuf_tensor(f"lt{i}", [P, XG * P], bf16_dt))
            for i in range(2)
        ]
        hb = [
            es.enter_context(nc.sbuf_tensor(f"hb{i}", [P, XG * D2], bf16_dt))
            for i in range(2)
        ]
        gat = [
            es.enter_context(nc.sbuf_tensor(f"gat{i}", [P, G * D2], bf16_dt))
            for i in range(2)
        ]
        pbt = es.enter_context(nc.sbuf_tensor("pbt", [P, NQ * (RG // 16)], i16))
        ps = [
            es.enter_context(nc.psum_tensor(f"ps{i}", [P, D2], f32))
            for i in range(NPS)
        ]
        # One dedicated semaphore per logical DMA: DMAs sharing a semaphore
        # complete in arbitrary order, so intermediate cumulative waits are
        # racy.  With a private sem, "done" is just sem >= its total.
        s_x = [es.enter_context(nc.semaphore(f"s_x{g}")) for g in range(NG)]
        s_lt = [es.enter_context(nc.semaphore(f"s_lt{g}")) for g in range(NG)]
        s_h = [es.enter_context(nc.semaphore(f"s_h{g}")) for g in range(NG)]
        s_gat = [es.enter_context(nc.semaphore(f"s_gat{q}")) for q in range(NQ)]
        s_out = [es.enter_context(nc.semaphore(f"s_out{q}")) for q in range(NQ)]
        s_pb = es.enter_context(nc.semaphore("s_pb"))    # pb load (DMA, +16)
        s_mm = es.enter_context(nc.semaphore("s_mm"))    # matmuls (PE, +1)
        s_chain = es.enter_context(nc.semaphore("s_chain"))  # state row (ACT, +1)
        s_hb = es.enter_context(nc.semaphore("s_hb"))    # hb copies (ACT, +1)
        block = es.enter_context(nc.Block())
        rhs0 = rhs[0]

        # A logical dma_start may be split into several InstDMACopy, each
        # incrementing the completion semaphore by 16.  Track the total per
        # semaphore so waits target the real completion value.
        sem_total: dict[str, int] = {}

        def count_dma(emit, sem) -> int:
            emit().then_inc(sem, 16)
            total = 0
            for inst in nc.all_instructions():
                si = inst.sync_info
                for u in si.on_update if si else []:
                    if u.ant_name == sem.name:
                        total += u.update_value
            sem_total[sem.name] = total
            return total

        def wait_done(engine, sem):
            engine.wait_ge(sem, sem_total[sem.name])

        lt_done: dict[int, int] = {}
        h_done: dict[int, int] = {}
        x_done: dict[int, int] = {}
        gat_done: dict[int, int] = {}
        out_done: dict[int, int] = {}
        pb_done: list[int] = []

        @block.sync
        def _(sp):
            def load_lt(g):
                lt_done[g] = count_dma(
                    lambda: sp.dma_start(
                        out=lt[g % 2][:, : _ng(g) * P].rearrange(
                            "p (c m) -> p c m", c=_ng(g)
                        ),
                        in_=lt_in[g * XG : g * XG + _ng(g)].rearrange(
                            "c k m -> k c m"
                        ),
                    ),
                    s_lt[g],
                )

            pb_done.append(count_dma(
                lambda: sp.dma_start(out=pbt[:], in_=pb_in[:]), s_pb
            ))
            # lt loads run two groups ahead; h-writes follow each hb group
            for g in range(min(2, NG)):
                load_lt(g)
            for g in range(NG):
                ng = _ng(g)
                # hb copies of group g done (implies group-g matmuls done,
                # so lt/rhs slot g%2 is also free for group g+2)
                sp.wait_ge(s_hb, g * XG + ng)
                h_done[g] = count_dma(
                    lambda: sp.dma_start(
                        out=h_dram[g * XG * C : (g * XG + ng) * C, :].rearrange(
                            "(c p) d -> p c d", p=C
                        ),
                        in_=hb[g % 2][1:P, : ng * D2].rearrange(
                            "p (c d) -> p c d", c=ng
                        ),
                    ),
                    s_h[g],
                )
                if g + 2 < NG:
                    load_lt(g + 2)

        @block.gpsimd
        def _(pool):
            from concourse import library_config

            # dma_gather's Q7 handler lives in the 'mlp' ucode library
            pool.load_library(library_config.mlp)
            h_waited = [False] * NG

            def load_x(g):
                if g >= NG:
                    return
                if g >= 2:
                    # slot g%2 free once group g-2 matmuls are done
                    pool.wait_ge(s_mm, (g - 1) * XG)
                x_done[g] = count_dma(
                    lambda: pool.dma_start(
                        out=rhs[g % 2][1:P, : _ng(g) * D2].rearrange(
                            "p (c d) -> p c d", c=_ng(g)
                        ),
                        in_=x_in[g * XG : g * XG + _ng(g)].rearrange(
                            "c p d -> p c d"
                        ),
                    ),
                    s_x[g],
                )

            def gather(q):
                if q == 0:
                    wait_done(pool, s_pb)
                if q >= 2:
                    # gat slot q%2 free once out-write q-2 is done
                    wait_done(pool, s_out[q - 2])
                for g in range(_n_hw(q)):
                    if not h_waited[g]:
                        wait_done(pool, s_h[g])
                        h_waited[g] = True
                bound = min((q + 1) * RG, MP)
                # gathered row i of this call -> tile [i%128, i//128, :]
                gat_done[q] = count_dma(
                    lambda: pool.dma_gather(
                        out_ap=gat[q % 2][:].rearrange("p (g d) -> p g d", g=G),
                        in_ap=h_dram[0:bound, :],
                        idxs_ap=pbt[:, q * (RG // 16) : (q + 1) * (RG // 16)],
                        num_idxs=RG,
                        num_idxs_reg=RG,
                        elem_size=D2,
                    ),
                    s_gat[q],
                )

            def write_out(q):
                wait_done(pool, s_gat[q])
                # widening cast bf16 -> f32 happens inside this SWDGE DMA;
                # gathered row i = output row q*RG + i with i = g*128 + p
                out_done[q] = count_dma(
                    lambda: pool.dma_start(
                        out=out[q * RG : (q + 1) * RG, :].rearrange(
                            "(g p) d -> p g d", p=P
                        ),
                        in_=gat[q % 2][:].rearrange("p (g d) -> p g d", g=G),
                    ),
                    s_out[q],
                )

            # interleave x loads with the expansion so gathers start as soon
            # as their h rows exist while x prefetch stays ahead of the scan
            for g in (0, 1, 2, 3):
                load_x(g)
            gather(0)
            load_x(4)
            write_out(0)
            gather(1)
            load_x(5)
            load_x(6)
            write_out(1)
            gather(2)
            load_x(7)
            load_x(8)
            write_out(2)
            for q in range(3, NQ):
                gather(q)
                write_out(q)

        @block.scalar
        def _(act):
            # initial scan state = 0 (chunk 0, rhs slot 0, block 0, row 0;
            # disjoint partitions from the x cast-DMA, so no ordering needed)
            nc.scalar.memzero(rhs0[0:1, 0:D2]).then_inc(s_chain, 1)
            for c in range(NCH):
                gi, g = c % XG, c // XG
                act.wait_ge(s_mm, c + 1)
                if c + 1 < NCH:
                    ngi, ng2 = (c + 1) % XG, (c + 1) // XG
                    nc.scalar.copy(
                        out=rhs[ng2 % 2][0:1, ngi * D2 : (ngi + 1) * D2],
                        in_=ps[c % NPS][0:1, :],
                    ).then_inc(s_chain, 1)
                if gi == 0 and g >= 2:
                    # hb slot g%2 free once h-write of group g-2 is done
                    act.wait_ge(s_h[g - 2], h_done[g - 2])
                nc.scalar.copy(
                    out=hb[g % 2][:, gi * D2 : (gi + 1) * D2],
                    in_=ps[c % NPS][:],
                ).then_inc(s_hb, 1)

        @block.tensor
        def _(pe):
            for c in range(NCH):
                gi, g = c % XG, c // XG
                if gi == 0:
                    pe.wait_ge(s_x[g], x_done[g])
                    pe.wait_ge(s_lt[g], lt_done[g])
                pe.wait_ge(s_chain, c + 1)
                if c >= NPS:
                    # PSUM slot free once its hb copy is done
                    pe.wait_ge(s_hb, c - NPS + 1)
                nc.tensor.matmul(
                    ps[c % NPS][:],
                    lhsT=lt[g % 2][:, gi * P : (gi + 1) * P],
                    rhs=rhs[g % 2][:, gi * D2 : (gi + 1) * D2],
                    start=True,
                    stop=True,
                ).then_inc(s_mm, 1)

    return nc


_program_cache: bass.Bass | None = None


def _get_program() -> bass.Bass:
    global _program_cache
    if _program_cache is None:
        _program_cache = _build_program()
    return _program_cache


def _prep_batch(bprob_b: np.ndarray, bmask_b: np.ndarray) -> dict[str, np.ndarray]:
    """Host-side index/scalar prep shared by the two cores of one batch."""
    idx = np.flatnonzero(bmask_b)
    K = int(min(idx.size, M))
    p = np.full(MP, 0.5, np.float32)
    p[:K] = np.clip(bprob_b[idx[:K], -1].astype(np.float32), EPS, 1.0 - EPS)
    # dt exactly as reference: f32(bf16(log(1/(1-p)))) with f32 arithmetic
    recip = np.float32(1.0) / (np.float32(1.0) - p)
    dt_f = np.log(recip, dtype=np.float32).astype(bf16).astype(np.float32)
    pw = np.where(np.arange(MP) < K, p, np.float32(0.0)).astype(np.float32)

    dtc = dt_f.reshape(NCH, C).astype(np.float64)
    S = np.cumsum(dtc, axis=1)  # [NCH, C] inclusive prefix sums
    diff = S[:, :, None] - S[:, None, :]  # [c, j, m] = S_j - S_m
    tril = np.arange(C)[:, None] <= np.arange(C)[None, :]
    # contraction row 0 = h_in slot, rows 1+j = tokens; output column 0 =
    # chunk-final state (dup of last token's column), columns 1+m = h_m
    lhsT = np.zeros((NCH, P, P), np.float32)
    lhsT[:, 1:, 1:] = (
        pw.reshape(NCH, C)[:, :, None] * np.exp(np.minimum(diff, 0.0)) * tril
    ).astype(np.float32)
    lhsT[:, 0, 1:] = np.exp(-S).astype(np.float32)  # h_in coefficients
    lhsT[:, :, 0] = lhsT[:, :, C]  # state output column (dup of last token col)
    lt = np.ascontiguousarray(lhsT.astype(bf16))

    cs = np.cumsum(bmask_b.astype(np.int64)) - 1
    pb = np.clip(cs, 0, M - 1).astype(np.int16)
    # dma_gather index layout: position i -> [i%16, i//16], replicated into
    # all 8 Q7-core stripes of 16 partitions
    per_q = pb.reshape(NQ, RG // 16, 16).transpose(0, 2, 1)  # [NQ, 16, RG//16]
    pb_t = np.tile(np.concatenate(list(per_q), axis=1), (P // 16, 1))
    return {"lt": lt, "pb": np.ascontiguousarray(pb_t)}


def _prep_inputs(
    hidden: np.ndarray, bprob: np.ndarray, bmask: np.ndarray
) -> list[dict[str, np.ndarray]]:
    in_maps = []
    per_batch = [_prep_batch(bprob[b], bmask[b]) for b in range(B)]
    for k in range(NCORES):
        b, half = divmod(k, 2)
        xh = np.zeros((MP, D2), np.float32)
        xh[:M] = hidden[b, :, half * D2 : (half + 1) * D2]
        m = dict(per_batch[b])
        m["x"] = np.ascontiguousarray(xh.reshape(NCH, C, D2))
        in_maps.append(m)
    return in_maps


def _run(in_maps, **kwargs):
    nc = _get_program()
    if not nc.is_finalized():
        nc.finalize()  # Bacc.finalize runs the full compile pipeline
    return run_bass_kernel_spmd(nc, in_maps, core_ids=list(range(NCORES)), **kwargs)


def kernel(**inputs: np.ndarray) -> np.ndarray:
    hidden = np.asarray(inputs["hidden_states"], dtype=np.float32)
    bprob = np.asarray(inputs["boundary_prob"], dtype=np.float32)
    bmask = np.asarray(inputs["boundary_mask"]).astype(bool)
    in_maps = _prep_inputs(hidden, bprob, bmask)
    res = _run(in_maps)
    out = np.empty((B, L, D), np.float32)
    for k in range(NCORES):
        b, half = divmod(k, 2)
        out[b, :, half * D2 : (half + 1) * D2] = res.results[k]["out"]
    return out


# revision 45
# speedup vs baseline: 1.4634x; 1.4634x over previous
"""DeChunk layer (ragged sequence scan + segment broadcast) on 8 trn2 NeuronCores.

Math (see reference): per batch b
  - boundary tokens (stable-sorted to front) give p_m for m < K
  - dt = bf16(log(1/(1-p))), x = bf16(hidden/dt)
  - linear recurrence h_m = exp(-dt_m) h_{m-1} + dt_m p_m x_m over the M sorted slots
  - out[l] = f32(bf16(h_{c(l)})), c(l) = cumsum(boundary_mask)-1  (segment broadcast)

Only scan positions m < K are ever read by the output, and the recurrence is
causal, so the tail is skipped.  The scan is reformulated as chunked matmuls
(Mamba-style): within a 127-token chunk with inclusive prefix sums S_m,
  h_m = exp(-S_m) h_in + sum_{j<=m} p_j exp(S_j - S_m) hidden_j
(the reference's w_j = dt_j p_j weight divided by its 1/dt_j x-scaling), which
is one [128,128]^T @ [128,512] matmul per chunk.  Contraction row 0 carries
h_in; lhsT column 0 duplicates the last token's column so the chunk-final
state lands on PSUM partition 0, partition-aligned with the next chunk's h_in
slot (engine ops cannot cross partitions and must start at partition 0).

Sharding: core k handles batch k//2, d_model half k%2 (512 columns).  Index
work (sort/cumsum/prefix sums/triangular exp matrices; all tiny) is done on
host; the device does all heavy data movement and math.

Device dataflow (one core), engine by engine:
  SP/HWDGE : x f32 loads, lhsT loads, h-store to DRAM.  All DRAM tensors are
             laid out partition-major on host so every DMA is 2D with >=1KB
             contiguous per-partition runs (scattered layouts measured 12x
             slower on the h-store).
  Vector   : per-group f32->bf16 cast of x into the matmul rhs tiles.
  Tensor   : 33 chunk matmuls.
  Scalar   : carried-state chain copy (PSUM row 0 -> next rhs row 0) and
             h copy (PSUM -> bf16 staging; this is the bf16 quantization).
  GpSimd   : dma_gather (the segment broadcast: 1024 rows per call, indices
             host-permuted so the following store is 128x16KB descriptors)
             and the widening bf16->f32 casting store to the output.

Written in raw Bacc (explicit semaphores, no TileContext): this container's
walrus codegen supports at most one attached sync-wait per compute
instruction, which rules out Tile's scheduler; raw Bass emits waits as
standalone instructions.  One semaphore per logical DMA (completion order
across DMAs sharing a semaphore is undefined).
"""

import numpy as np
import ml_dtypes

import concourse.bass as bass
import concourse.bacc as bacc
import concourse.mybir as mybir
from concourse.bass_utils import run_bass_kernel_spmd

EPS = 1e-4
B, L, M, D = 4, 8192, 4096, 1024
NCORES = 8
D2 = D // 2            # columns per core
P = 128
C = 127                # tokens per scan chunk (slot 0 = carried state)
NCH = (M + C - 1) // C # 33
MP = NCH * C           # 4191
XG = 4                 # scan chunks per load/store group
NG = (NCH + XG - 1) // XG  # 9
NQ = 8                 # output expansion calls
RG = L // NQ           # 1024 output rows per call
G = RG // P            # 8 rows per partition per call
NPS = 4                # PSUM buffers

bf16 = ml_dtypes.bfloat16
f32 = mybir.dt.float32
bf16_dt = mybir.dt.bfloat16
i16 = mybir.dt.int16


def _ng(g: int) -> int:
    return min(XG, NCH - g * XG)


def _n_hw(q: int) -> int:
    """h-store groups required before output-gather q (c(l) <= l bound)."""
    bound = min((q + 1) * RG, MP)
    return -(-bound // (XG * C))


def _build_program() -> bass.Bass:
    # Bacc (not plain Bass): its compile() pipeline lowers the gpsimd
    # library-reload pseudo-instruction and custom-ISA ops for walrus
    nc = bacc.Bacc("TRN2")
    # partition-major: x[1+j, c*D2 + d] = hidden row 127c + j; row 0 is a
    # host-zeroed dummy so loads cover all 128 partitions (the rhs row-0
    # state slot is overwritten by the chain copy anyway)
    x_in = nc.declare_dram_parameter("x", [P, NCH * D2], f32, isOutput=False)
    # lt[k, c*P + m]: per-chunk stationary operands, partition-major
    lt_in = nc.declare_dram_parameter("lt", [P, NCH * P], bf16_dt, isOutput=False)
    # int16 gather indices (dma_gather wrapped-by-16 layout, host-prepared)
    pb_in = nc.declare_dram_parameter("pb", [P, NQ * (RG // 16)], i16, isOutput=False)
    out = nc.declare_dram_parameter("out", [L, D2], f32, isOutput=True)
    # partition-major h bounce: h[j, c*D2 + d] = h_{127c + j}
    h_dram = nc.dram_tensor("h_dram", [C, NCH * D2], bf16_dt)

    from contextlib import ExitStack

    with ExitStack() as es:
        xf = [
            es.enter_context(nc.sbuf_tensor(f"xf{i}", [P, XG * D2], f32))
            for i in range(2)
        ]
        rhs = [
            es.enter_context(nc.sbuf_tensor(f"rhs{i}", [P, XG * D2], bf16_dt))
            for i in range(2)
        ]
        lt = [
            es.enter_context(nc.sbuf_tensor(f"lt{i}", [P, XG * P], bf16_dt))
            for i in range(2)
        ]
        hb = [
            es.enter_context(nc.sbuf_tensor(f"hb{i}", [P, XG * D2], bf16_dt))
            for i in range(2)
        ]
        gat = [
            es.enter_context(nc.sbuf_tensor(f"gat{i}", [P, G * D2], bf16_dt))
            for i in range(2)
        ]
        pbt = es.enter_context(nc.sbuf_tensor("pbt", [P, NQ * (RG // 16)], i16))
        ps = [
            es.enter_context(nc.psum_tensor(f"ps{i}", [P, D2], f32))
            for i in range(NPS)
        ]
        # One dedicated semaphore per logical DMA / per group event
        s_x = [es.enter_context(nc.semaphore(f"s_x{g}")) for g in range(NG)]
        s_lt = [es.enter_context(nc.semaphore(f"s_lt{g}")) for g in range(NG)]
        s_h = [es.enter_context(nc.semaphore(f"s_h{g}")) for g in range(NG)]
        s_cv = [es.enter_context(nc.semaphore(f"s_cv{g}")) for g in range(NG)]
        s_gat = [es.enter_context(nc.semaphore(f"s_gat{q}")) for q in range(NQ)]
        s_out = [es.enter_context(nc.semaphore(f"s_out{q}")) for q in range(NQ)]
        s_pb = es.enter_context(nc.semaphore("s_pb"))    # pb load (DMA, +16)
        s_mm = es.enter_context(nc.semaphore("s_mm"))    # matmuls (PE, +1)
        s_chain = es.enter_context(nc.semaphore("s_chain"))  # state row (ACT, +1)
        s_hb = es.enter_context(nc.semaphore("s_hb"))    # hb copies (ACT, +1)
        block = es.enter_context(nc.Block())

        # A logical dma_start may lower to several InstDMACopy, each
        # incrementing the completion semaphore by 16.  Track the total per
        # semaphore so waits target the real completion value.
        sem_total: dict[str, int] = {}

        def count_dma(emit, sem) -> int:
            emit().then_inc(sem, 16)
            total = 0
            for inst in nc.all_instructions():
                si = inst.sync_info
                for u in si.on_update if si else []:
                    if u.ant_name == sem.name:
                        total += u.update_value
            sem_total[sem.name] = total
            return total

        def wait_done(engine, sem):
            engine.wait_ge(sem, sem_total[sem.name])

        @block.sync
        def _(sp):
            def load_x(g):
                count_dma(
                    lambda: sp.dma_start(
                        out=xf[g % 2][:, : _ng(g) * D2],
                        in_=x_in[:, g * XG * D2 : (g * XG + _ng(g)) * D2],
                    ),
                    s_x[g],
                )

            def load_lt(g):
                count_dma(
                    lambda: sp.dma_start(
                        out=lt[g % 2][:, : _ng(g) * P],
                        in_=lt_in[:, g * XG * P : (g * XG + _ng(g)) * P],
                    ),
                    s_lt[g],
                )

            count_dma(lambda: sp.dma_start(out=pbt[:], in_=pb_in[:]), s_pb)
            for g in range(min(2, NG)):
                load_x(g)
                load_lt(g)
            for g in range(NG):
                ng = _ng(g)
                # hb copies of group g done; this also implies group-g
                # matmuls, the group-g rhs cast, and lt reads are done, so
                # the g+2 loads below can safely reuse slot g%2
                sp.wait_ge(s_hb, g * XG + ng)
                count_dma(
                    lambda: sp.dma_start(
                        out=h_dram[:, g * XG * D2 : (g * XG + ng) * D2],
                        in_=hb[g % 2][1:P, : ng * D2],
                    ),
                    s_h[g],
                )
                if g + 2 < NG:
                    load_x(g + 2)
                    load_lt(g + 2)

        @block.vector
        def _(dve):
            for g in range(NG):
                wait_done(dve, s_x[g])
                if g >= 2:
                    dve.wait_ge(s_mm, (g - 1) * XG)  # rhs slot g%2 free
                nc.vector.tensor_copy(
                    out=rhs[g % 2][:, : _ng(g) * D2],
                    in_=xf[g % 2][:, : _ng(g) * D2],
                ).then_inc(s_cv[g], 1)

        @block.scalar
        def _(act):
            # initial scan state = 0 (chunk 0, rhs slot 0, block 0, row 0);
            # must follow the group-0 cast, which fills row 0 with garbage
            act.wait_ge(s_cv[0], 1)
            nc.scalar.memzero(rhs[0][0:1, 0:D2]).then_inc(s_chain, 1)
            for c in range(NCH):
                gi, g = c % XG, c // XG
                act.wait_ge(s_mm, c + 1)
                if c + 1 < NCH:
                    ngi, ng2 = (c + 1) % XG, (c + 1) // XG
                    if ngi == 0:
                        # first state write into group ng2: the group cast
                        # must have happened (it clobbers row 0)
                        act.wait_ge(s_cv[ng2], 1)
                    nc.scalar.copy(
                        out=rhs[ng2 % 2][0:1, ngi * D2 : (ngi + 1) * D2],
                        in_=ps[c % NPS][0:1, :],
                    ).then_inc(s_chain, 1)
                if gi == 0 and g >= 2:
                    # hb slot g%2 free once h-store of group g-2 is done
                    act.wait_ge(s_h[g - 2], sem_total[s_h[g - 2].name])
                nc.scalar.copy(
                    out=hb[g % 2][:, gi * D2 : (gi + 1) * D2],
                    in_=ps[c % NPS][:],
                ).then_inc(s_hb, 1)

        @block.tensor
        def _(pe):
            for c in range(NCH):
                gi, g = c % XG, c // XG
                if gi == 0:
                    pe.wait_ge(s_lt[g], sem_total[s_lt[g].name])
                # s_chain covers the rhs transitively: the chain copy for
                # chunk c was emitted after ACT waited on the group cast
                pe.wait_ge(s_chain, c + 1)
                if c >= NPS:
                    # PSUM slot free once its hb copy is done
                    pe.wait_ge(s_hb, c - NPS + 1)
                nc.tensor.matmul(
                    ps[c % NPS][:],
                    lhsT=lt[g % 2][:, gi * P : (gi + 1) * P],
                    rhs=rhs[g % 2][:, gi * D2 : (gi + 1) * D2],
                    start=True,
                    stop=True,
                ).then_inc(s_mm, 1)

        @block.gpsimd
        def _(pool):
            from concourse import library_config

            # dma_gather's Q7 handler lives in the 'mlp' ucode library
            pool.load_library(library_config.mlp)
            h_waited = [False] * NG

            def gather(q):
                if q == 0:
                    wait_done(pool, s_pb)
                if q >= 2:
                    # gat slot q%2 free once out-store q-2's read is done
                    wait_done(pool, s_out[q - 2])
                for g in range(_n_hw(q)):
                    if not h_waited[g]:
                        wait_done(pool, s_h[g])
                        h_waited[g] = True
                # gathered position i -> tile [i%128, i//128, :]; indices are
                # host-permuted so i = g*128 + p holds output row p*G + g
                count_dma(
                    lambda: pool.dma_gather(
                        out_ap=gat[q % 2][:].rearrange("p (g d) -> p g d", g=G),
                        in_ap=h_dram[:].rearrange("p (c d) -> (p c) d", d=D2),
                        idxs_ap=pbt[:, q * (RG // 16) : (q + 1) * (RG // 16)],
                        num_idxs=RG,
                        num_idxs_reg=RG,
                        elem_size=D2,
                    ),
                    s_gat[q],
                )

            def write_out(q):
                wait_done(pool, s_gat[q])
                # widening cast bf16 -> f32 inside this SWDGE DMA; the host
                # index permutation makes each partition's 8 rows consecutive
                # in DRAM (128 x 16KB descriptors)
                count_dma(
                    lambda: pool.dma_start(
                        out=out[q * RG : (q + 1) * RG, :].rearrange(
                            "(p g) d -> p (g d)", p=P
                        ),
                        in_=gat[q % 2][:],
                    ),
                    s_out[q],
                )

            for q in range(NQ):
                gather(q)
                write_out(q)

    return nc


_program_cache: bass.Bass | None = None


def _get_program() -> bass.Bass:
    global _program_cache
    if _program_cache is None:
        _program_cache = _build_program()
    return _program_cache


def _prep_batch(bprob_b: np.ndarray, bmask_b: np.ndarray) -> dict[str, np.ndarray]:
    """Host-side index/scalar prep shared by the two cores of one batch."""
    idx = np.flatnonzero(bmask_b)
    K = int(min(idx.size, M))
    p = np.full(MP, 0.5, np.float32)
    p[:K] = np.clip(bprob_b[idx[:K], -1].astype(np.float32), EPS, 1.0 - EPS)
    # dt exactly as reference: f32(bf16(log(1/(1-p)))) with f32 arithmetic
    recip = np.float32(1.0) / (np.float32(1.0) - p)
    dt_f = np.log(recip, dtype=np.float32).astype(bf16).astype(np.float32)
    pw = np.where(np.arange(MP) < K, p, np.float32(0.0)).astype(np.float32)

    dtc = dt_f.reshape(NCH, C).astype(np.float64)
    S = np.cumsum(dtc, axis=1)  # [NCH, C] inclusive prefix sums
    diff = S[:, :, None] - S[:, None, :]  # [c, j, m] = S_j - S_m
    tril = np.arange(C)[:, None] <= np.arange(C)[None, :]
    # contraction row 0 = h_in slot, rows 1+j = tokens; output column 0 =
    # chunk-final state (dup of last token's column), columns 1+m = h_m
    lhsT = np.zeros((NCH, P, P), np.float32)
    lhsT[:, 1:, 1:] = (
        pw.reshape(NCH, C)[:, :, None] * np.exp(np.minimum(diff, 0.0)) * tril
    ).astype(np.float32)
    lhsT[:, 0, 1:] = np.exp(-S).astype(np.float32)  # h_in coefficients
    lhsT[:, :, 0] = lhsT[:, :, C]  # state output column (dup of last token col)
    # partition-major for clean 2D loads: lt2[k, c*P + m]
    lt2 = np.ascontiguousarray(
        lhsT.astype(bf16).transpose(1, 0, 2).reshape(P, NCH * P)
    )

    cs = np.cumsum(bmask_b.astype(np.int64)) - 1
    pb = np.clip(cs, 0, M - 1)
    # remap into h_dram's partition-major row space ...
    r = ((pb % C) * NCH + pb // C).astype(np.int16)
    # ... permute so gather position i = g*128+p serves output row p*G+g ...
    rq = r.reshape(NQ, P, G).transpose(0, 2, 1).reshape(NQ, RG)
    # ... and wrap by 16 (position i -> [i%16, i//16]), replicated to all
    # eight 16-partition Q7-core stripes
    per_q = rq.reshape(NQ, RG // 16, 16).transpose(0, 2, 1)  # [NQ, 16, RG/16]
    pb_t = np.tile(np.concatenate(list(per_q), axis=1), (P // 16, 1))
    return {"lt": lt2, "pb": np.ascontiguousarray(pb_t)}


def _prep_inputs(
    hidden: np.ndarray, bprob: np.ndarray, bmask: np.ndarray
) -> list[dict[str, np.ndarray]]:
    in_maps = []
    per_batch = [_prep_batch(bprob[b], bmask[b]) for b in range(B)]
    for k in range(NCORES):
        b, half = divmod(k, 2)
        xh = np.zeros((MP, D2), np.float32)
        xh[:M] = hidden[b, :, half * D2 : (half + 1) * D2]
        m = dict(per_batch[b])
        # partition-major with zero dummy row 0: x2[1+j, c*D2+d] = xh[127c+j]
        x2 = np.zeros((P, NCH * D2), np.float32)
        x2[1:] = xh.reshape(NCH, C, D2).transpose(1, 0, 2).reshape(C, NCH * D2)
        m["x"] = x2
        in_maps.append(m)
    return in_maps


def _run(in_maps, **kwargs):
    nc = _get_program()
    if not nc.is_finalized():
        nc.finalize()  # Bacc.finalize runs the full compile pipeline
    return run_bass_kernel_spmd(nc, in_maps, core_ids=list(range(NCORES)), **kwargs)


def kernel(**inputs: np.ndarray) -> np.ndarray:
    hidden = np.asarray(inputs["hidden_states"], dtype=np.float32)
    bprob = np.asarray(inputs["boundary_prob"], dtype=np.float32)
    bmask = np.asarray(inputs["boundary_mask"]).astype(bool)
    in_maps = _prep_inputs(hidden, bprob, bmask)
    res = _run(in_maps)
    out = np.empty((B, L, D), np.float32)
    for k in range(NCORES):
        b, half = divmod(k, 2)
        out[b, :, half * D2 : (half + 1) * D2] = res.results[k]["out"]
    return out
